# revision 20
# baseline (speedup 1.0000x reference)
"""GCNConv (add self-loops, symmetric norm, linear, relu, broadcast) on 8 TRN2 cores.

Hybrid dense/sparse aggregation, no cross-core communication:

- Destination nodes are row-sharded across the 8 cores (1250 rows each).
- Phase 0 (replicated): every core computes h = x @ W (x supplied
  pre-transposed/padded).  The first NBS source blocks ("sparse range",
  split into STAGES) are written to per-stage DRAM tensors; the last
  D_DENSE blocks ("dense range") stay resident in SBUF.
- Phase 1, per 128-row destination tile:
    sparse range: host-deduplicated source rows are gathered per
      (tile, stage) with dma_gather — staging lets gathers for early
      stages overlap the rest of phase 0 (the Q7 descriptor-emission
      engine is the sparse path's bottleneck, so it must start early) —
      and reduced with PE matmuls against host-built scatter blocks
      S[u, d] = sum of edge norms from gathered slot u into dst d.
    dense range: a host-built dense block adjacency A[s, d] is streamed
      from DRAM (HWDGE, no per-row descriptors) and reduced with PE
      matmuls against the SBUF-resident h blocks.
  Both accumulate into the same PSUM tile; bias-add + relu on DVE.

The dense/sparse split ratio balances the PE (matmul) against the Q7
(SWDGE descriptor emission, ~7 ns/row hard floor) and HBM, which
otherwise bottleneck the all-sparse / all-dense variants respectively.
dma_gather is limited to 1024 indices per call.
"""

import numpy as np

import concourse.bacc as bacc
import concourse.mybir as mybir
import concourse.tile as tile
from concourse.bass_utils import run_bass_kernel_spmd

N_NODES = 10000
N_GENES = 978
EMBED = 301
HEADS = 4
REP = 12
N_CORES = 8
NPC = N_NODES // N_CORES          # 1250 dst rows per core
DT = 128                          # dst tile height
NT = (NPC + DT - 1) // DT         # 10 dst tiles per core
GP = 1024                         # padded gene dim (8 chunks of 128)
GCH = GP // 128
NB = 79                           # src blocks of 128 (79*128 = 10112 >= 10000)
SP = NB * 128
D_DENSE = 40                      # dense src blocks (tail of the range)
NBS = NB - D_DENSE                # sparse src blocks
SPR = NBS * 128                   # sparse rows
# sparse-range stage cuts (in blocks); stage s covers [CUTS[s], CUTS[s+1])
CUTS = [0, 8, 18, 28, NBS]
NST = len(CUTS) - 1
G_BUFS = 6                        # gather tiles in flight per stage
HROW = 384                        # gather elem: 384 f16 = 768 B (mult of 256)
SG = 512                          # xT stream chunk (free dim)

F32 = mybir.dt.float32
F16 = mybir.dt.float16
I16 = mybir.dt.int16

_prog_cache: dict = {}


def _build_program(bmaxs: tuple):
    offs = [0]
    for s in range(NST):
        offs.append(offs[-1] + bmaxs[s])
    bmax = offs[-1]               # total gather blocks per tile
    slots = bmax * 128
    nc = bacc.Bacc("TRN2", target_bir_lowering=False, debug=False,
                   num_devices=N_CORES, num_swdge_queues=4)

    xT_d = nc.dram_tensor("xT", [GP, SP], F16, kind="ExternalInput")
    W_d = nc.dram_tensor("Wp", [GP, EMBED], F16, kind="ExternalInput")
    b_d = nc.dram_tensor("bB", [128, EMBED], F32, kind="ExternalInput")
    S_d = nc.dram_tensor("Sblk", [NT, 128, slots], F16, kind="ExternalInput")
    ix_d = nc.dram_tensor("idxw", [NT, 128, slots // 16], I16, kind="ExternalInput")
    A_d = nc.dram_tensor("Adns", [NT, 128, D_DENSE, 128], F16, kind="ExternalInput")
    out_d = nc.dram_tensor("out", [NPC, EMBED], F16, kind="ExternalOutput")
    # per-stage sparse h tensors: precise per-stage RAW tracking
    hst_d = [nc.dram_tensor(f"hst{s}", [(CUTS[s + 1] - CUTS[s]) * 128, HROW], F16)
             for s in range(NST)]

    with tile.TileContext(nc) as tc:
        with (
            tc.tile_pool(name="const", bufs=1) as cpool,
            tc.tile_pool(name="sI", bufs=NT) as ipool,
            tc.tile_pool(name="sS", bufs=2) as spool,
            tc.tile_pool(name="sA", bufs=2) as apool,
            tc.tile_pool(name="sO", bufs=2) as opool,
            tc.tile_pool(name="pO", bufs=4, space="PSUM") as popool,
            tc.tile_pool(name="gp0", bufs=G_BUFS) as gpool0,
            tc.tile_pool(name="gp1", bufs=G_BUFS) as gpool1,
            tc.tile_pool(name="gp2", bufs=G_BUFS) as gpool2,
            tc.tile_pool(name="gp3", bufs=G_BUFS) as gpool3,
        ):
            gpools = [gpool0, gpool1, gpool2, gpool3]
            b_sb = cpool.tile([128, EMBED], F32, tag="bias")
            nc.sync.dma_start(b_sb[:], b_d[:])
            hres = cpool.tile([128, D_DENSE, EMBED], F16, tag="hres")

            ix_sbs = []
            for t in range(NT):
                ix_sb = ipool.tile([128, slots // 16], I16, tag="ix")
                nc.sync.dma_start(ix_sb[:], ix_d[t])
                ix_sbs.append(ix_sb)

            # ---------------- phase 0: h = x @ W ----------------
            sents = [None] * NST
            with (
                tc.tile_pool(name="wsb", bufs=1) as wpool,
                tc.tile_pool(name="xt", bufs=2) as xpool,
                tc.tile_pool(name="hsb", bufs=4) as hpool,
                tc.tile_pool(name="ph", bufs=4, space="PSUM") as phpool,
            ):
                w_sb = wpool.tile([128, GCH, EMBED], F16)
                for g in range(GCH):
                    nc.sync.dma_start(w_sb[:, g, :], W_d[g * 128:(g + 1) * 128, :])

                h_writes = [[] for _ in range(NST)]
                for s0 in range(0, SP, SG):
                    sgw = min(SG, SP - s0)
                    xt = xpool.tile([128, GCH, SG], F16, tag="xt")
                    nc.sync.dma_start(xt[:, :, :sgw],
                                      xT_d[:].rearrange("(g p) n -> p g n", p=128)
                                      [:, :, s0:s0 + sgw])
                    for sub in range(sgw // 128):
                        blk = (s0 + sub * 128) // 128
                        ph = phpool.tile([128, EMBED], F32)
                        for g in range(GCH):
                            nc.tensor.matmul(
                                ph[:],
                                xt[:, g, sub * 128:(sub + 1) * 128],
                                w_sb[:, g, :],
                                start=(g == 0), stop=(g == GCH - 1),
                            )
                        if blk < NBS:
                            st = next(s for s in range(NST)
                                      if blk < CUTS[s + 1])
                            h_sb = hpool.tile([128, EMBED], F16, tag="h")
                            nc.vector.tensor_copy(h_sb[:], ph[:])
                            r = (blk - CUTS[st]) * 128
                            h_writes[st].append(nc.scalar.dma_start(
                                hst_d[st][r:r + 128, :EMBED], h_sb[:]))
                            if blk == CUTS[st + 1] - 1:
                                sent = nc.sync.nop()
                                for hw in h_writes[st]:
                                    tile.add_dep_helper(
                                        sent.ins, hw.ins,
                                        reason=f"h stage {st} ready")
                                sents[st] = sent
                        else:
                            nc.vector.tensor_copy(hres[:, blk - NBS, :], ph[:])

            # sparse gathers: stage-major so stage-0 gathers (all tiles)
            # run while phase 0 is still producing later stages
            g_sbs = {}  # (t, s) -> tile
            qctr = 0
            for s in range(NST):
                for t in range(NT):
                    g_sb = gpools[s].tile([128, bmaxs[s], HROW], F16,
                                          tag=f"g{s}")
                    gi = nc.gpsimd.dma_gather(
                        g_sb[:], hst_d[s][:],
                        ix_sbs[t][:, offs[s] * 8:offs[s + 1] * 8],
                        num_idxs=bmaxs[s] * 128, num_idxs_reg=bmaxs[s] * 128,
                        elem_size=HROW, queue_num=0,
                    )
                    qctr += 1
                    tile.add_dep_helper(gi.ins, sents[s].ins,
                                        reason=f"gather waits h stage {s}")
                    g_sbs[(t, s)] = g_sb

            # ------------- phase 1: S-matmul + dense + bias/relu -------
            pre_s = {}
            pre_a = {}
            for t in range(2):
                s_sb = spool.tile([128, slots], F16, tag="s")
                nc.sync.dma_start(s_sb[:], S_d[t])
                pre_s[t] = s_sb
                a_sb = apool.tile([128, D_DENSE, 128], F16, tag="a")
                nc.sync.dma_start(a_sb[:], A_d[t])
                pre_a[t] = a_sb

            for t in range(NT):
                r0 = t * DT
                nr = min(DT, NPC - r0)
                s_sb = pre_s.pop(t)
                a_sb = pre_a.pop(t)
                tn = t + 2
                if tn < NT:
                    s_nx = spool.tile([128, slots], F16, tag="s")
                    nc.sync.dma_start(s_nx[:], S_d[tn])
                    pre_s[tn] = s_nx
                    a_nx = apool.tile([128, D_DENSE, 128], F16, tag="a")
                    nc.sync.dma_start(a_nx[:], A_d[tn])
                    pre_a[tn] = a_nx

                po = popool.tile([128, EMBED], F32)
                first = True
                for s in range(NST):
                    g_sb = g_sbs.pop((t, s))
                    for j in range(bmaxs[s]):
                        nc.tensor.matmul(
                            po[:],
                            s_sb[:, (offs[s] + j) * 128:(offs[s] + j + 1) * 128],
                            g_sb[:, j, :EMBED],
                            start=first, stop=False,
                        )
                        first = False
                for j in range(D_DENSE):
                    nc.tensor.matmul(
                        po[:],
                        a_sb[:, j, :],
                        hres[:, j, :],
                        start=False, stop=(j == D_DENSE - 1),
                    )
                o_sm = opool.tile([128, EMBED], F32, tag="osm")
                nc.vector.tensor_add(o_sm[:], po[:], b_sb[:])
                nc.vector.tensor_relu(o_sm[:], o_sm[:])
                o_cast = opool.tile([128, EMBED], F16, tag="ocast")
                nc.vector.tensor_copy(o_cast[:], o_sm[:])
                nc.scalar.dma_start(out_d[r0:r0 + nr, :], o_cast[:nr, :])

    nc.compile()
    return nc


def _preprocess(x, edge_index, edge_weight, W, b):
    src = np.concatenate([edge_index[0].astype(np.int64),
                          np.arange(N_NODES, dtype=np.int64)])
    dst = np.concatenate([edge_index[1].astype(np.int64),
                          np.arange(N_NODES, dtype=np.int64)])
    wf = np.concatenate([edge_weight.astype(np.float32),
                         np.ones(N_NODES, np.float32)])

    deg = np.bincount(dst, weights=wf.astype(np.float64),
                      minlength=N_NODES).astype(np.float32)
    dis = np.where(deg > 0, 1.0 / np.sqrt(deg), 0.0).astype(np.float32)
    norm = (dis[src] * wf * dis[dst]).astype(np.float32)

    order = np.argsort(dst, kind="stable")
    src_s, dst_s, norm_s = src[order], dst[order], norm[order]

    core_of = dst_s // NPC
    tloc_of = (dst_s % NPC) // DT
    group = core_of * NT + tloc_of
    cnt = np.bincount(group, minlength=N_CORES * NT)
    gstart = np.zeros(N_CORES * NT + 1, np.int64)
    gstart[1:] = np.cumsum(cnt)
    dloc = (dst_s % NPC) % DT

    # sparse range: dedup per (core, tile, stage); dense range: raw edges
    uniq = []
    max_u = [0] * NST
    for g in range(N_CORES * NT):
        lo, hi = gstart[g], gstart[g + 1]
        sg, dg, ng = src_s[lo:hi], dloc[lo:hi], norm_s[lo:hi]
        per_stage = []
        for s in range(NST):
            m = (sg >= CUTS[s] * 128) & (sg < CUTS[s + 1] * 128)
            u, inv = np.unique(sg[m], return_inverse=True)
            per_stage.append((u, inv, dg[m]))
            max_u[s] = max(max_u[s], len(u))
        md = sg >= SPR
        uniq.append((g // NT, g % NT, per_stage, lo, hi,
                     sg[md] - SPR, dg[md], ng[md]))
    bmaxs = tuple(max(1, (mu + 127) // 128) for mu in max_u)
    offs = [0]
    for s in range(NST):
        offs.append(offs[-1] + bmaxs[s])
    slots = offs[-1] * 128

    idx_arr = np.zeros((N_CORES, NT, slots), np.int16)
    S_f32 = np.zeros((N_CORES, NT, 128, slots), np.float32)
    A_f32 = np.zeros((N_CORES, NT, 128, D_DENSE, 128), np.float32)
    for k, t, per_stage, lo, hi, sd, dd, nd in uniq:
        sg, ng = src_s[lo:hi], norm_s[lo:hi]
        for s, (u, inv, dg) in enumerate(per_stage):
            o = offs[s] * 128
            idx_arr[k, t, o:o + len(u)] = (u - CUTS[s] * 128).astype(np.int16)
            m = (sg >= CUTS[s] * 128) & (sg < CUTS[s + 1] * 128)
            np.add.at(S_f32[k, t],
                      (inv % 128, (offs[s] + inv // 128) * 128 + dg), ng[m])
        np.add.at(A_f32[k, t], (sd % 128, sd // 128, dd), nd)
    S_arr = S_f32.astype(np.float16)
    A_arr = A_f32.astype(np.float16)

    # SWDGE index layout: idx i lives at (partition i%16, col i//16),
    # replicated across the 8 sixteen-partition groups.
    cols = np.arange(slots // 16)
    idx_w = np.empty((N_CORES, NT, 128, slots // 16), np.int16)
    for p in range(16):
        lane = idx_arr[:, :, cols * 16 + p]
        idx_w[:, :, p::16, :] = lane[:, :, None, :]

    xT = np.zeros((GP, SP), np.float16)
    xT[:N_GENES, :N_NODES] = np.ascontiguousarray(
        x.astype(np.float32).T).astype(np.float16)
    Wp = np.zeros((GP, EMBED), np.float16)
    Wp[:N_GENES] = W.astype(np.float32).astype(np.float16)
    bB = np.broadcast_to(b.astype(np.float32), (128, EMBED)).copy()
    return xT, Wp, bB, S_arr, idx_w, A_arr, bmaxs


def make_in_maps(x, edge_index, edge_weight, W, b):
    xT, Wp, bB, S_arr, idx_w, A_arr, bmaxs = _preprocess(
        x, edge_index, edge_weight, W, b)
    in_maps = [
        {"xT": xT, "Wp": Wp, "bB": bB, "Sblk": S_arr[k], "idxw": idx_w[k],
         "Adns": A_arr[k]}
        for k in range(N_CORES)
    ]
    return in_maps, bmaxs


def get_program(bmaxs):
    if bmaxs not in _prog_cache:
        _prog_cache[bmaxs] = _build_program(bmaxs)
    return _prog_cache[bmaxs]


def kernel(x, edge_index, edge_weight, W, b):
    x = np.asarray(x)
    edge_index = np.asarray(edge_index)
    edge_weight = np.asarray(edge_weight)
    W = np.asarray(W)
    b = np.asarray(b)

    in_maps, bmaxs = make_in_maps(x, edge_index, edge_weight, W, b)
    nc = get_program(bmaxs)
    res = run_bass_kernel_spmd(nc, in_maps, core_ids=list(range(N_CORES)))
    out = np.concatenate([res.results[k]["out"] for k in range(N_CORES)], axis=0)
    out = np.asarray(out, dtype=np.float32)  # [N_NODES, EMBED]
    # unsqueeze(1)/unsqueeze(3) + repeat is a pure broadcast: do it on host
    return np.broadcast_to(out[:, None, :, None],
                           (N_NODES, HEADS, EMBED, REP))
